# revision 1
# baseline (speedup 1.0000x reference)
"""Trainium2 Bass kernel: 3x3 stride-1 pad-1 conv, x(32,3,224,224) * W(64,3,3,3) + b -> (32,64,224,224).

Data-parallel over batch: 4 images per core on 8 cores.  Per core ("quad" design):
  - Each of the 4 images owns a 32-partition block (bases 0/32/64/96).  Within a
    block, 18 partitions hold the image in 6 shifted copies: partition
    32n + (g*3+dx)*3 + ci at padded position p = r*226+c contains
    xpad[n, ci, r+2g, c+dx]  (xpad = zero-padded 226x226 image).
  - One output chunk = 4 output rows of one image = psum[128, 452]:
      rows 0:64   = channels for output rows h..h+1   (block g=0)
      rows 64:128 = channels for output rows h+2..h+3 (block g=1)
    computed as 3 PSUM-accumulating matmuls (dy=0,1,2):
      lhsT = Wdy [18,128] block-diagonal at base 32n, rhs = S[32n:32n+18, ...],
      tile_position=(32n,0) -> the 4 images' matmuls run concurrently in
      different PE row-groups.
  - float32r everywhere on the matmul path (1 cycle/row at N>=256); x and
    weight DRAM tensors are declared float32r so all fills are cast-free and
    can issue from any DMA ring (sync/scalar HWDGE + gpsimd SWDGE).
  - PSUM -> SBUF staging copy adds bias (DVE tensor_scalar_add / ACT
    activation Identity, alternating), then large strided DMAs write HBM.
"""

import os
import sys

import numpy as np

for _p in ("/opt/trn_rl_repo", "/root/.axon_site/_ro/trn_rl_repo"):
    if os.path.isdir(_p) and _p not in sys.path:
        sys.path.insert(0, _p)

import concourse.bass as bass  # noqa: E402
import concourse.tile as tile  # noqa: E402
from concourse import bacc, mybir  # noqa: E402
from concourse.bass_utils import run_bass_kernel_spmd  # noqa: E402

# Problem constants (hardcoded per contract).
N, CIN, H, W = 32, 3, 224, 224
COUT, KK = 64, 3
NCORES = 8
NPER = N // NCORES          # 4 images per core = 4 PE row-groups
WP = W + 2                  # padded width: 226
SR = 56                     # slab rows
NSLAB = H // SR             # 4 slabs
CHROWS = 4                  # output rows per matmul chunk
NCOL = (CHROWS // 2) * WP   # 452: matmul moving free size
CPS = SR // CHROWS          # 14 chunk positions per slab
GROUPS = (4, 4, 4, 2)       # chunks per staging/output-DMA group (sum = CPS)

# dest column range [c0, c1) filled from x for each dx; rest is zero padding.
# dest col c <- x col (c + dx - 1).
ZCOLS = {0: (1, 225), 1: (0, 224), 2: (0, 223)}

F32 = mybir.dt.float32
F32R = mybir.dt.float32r
IDENT = mybir.ActivationFunctionType.Identity


class _RR:
    """Round-robin DMA issuing engine picker with weights."""

    def __init__(self, nc, pattern):
        self.engines = {"sync": nc.sync, "gpsimd": nc.gpsimd, "scalar": nc.scalar}
        self.pattern = pattern
        self.i = 0

    def __call__(self):
        e = self.engines[self.pattern[self.i % len(self.pattern)]]
        self.i += 1
        return e


def _emit(ctx, tc, o_ap, x_ap, w_ap, b_ap, repeat=1):
    nc = tc.nc

    wpool = ctx.enter_context(tc.tile_pool(name="wpool", bufs=1))
    spool = ctx.enter_context(tc.tile_pool(name="spool", bufs=1))
    ppool = ctx.enter_context(tc.tile_pool(name="ppool", bufs=8, space="PSUM"))
    stpool = ctx.enter_context(tc.tile_pool(name="stpool", bufs=8))

    dma_rr = _RR(nc, ("sync", "gpsimd", "sync", "gpsimd", "scalar"))

    # f32r zeros source (f32r may only be produced by DMA; one casting SWDGE
    # DMA from a memset f32 tile, then all zero-fills copy from it cast-free).
    zf = wpool.tile([128, 256], F32, name="zf", tag="zf")
    nc.vector.memset(zf[:], 0.0)
    zr = wpool.tile([128, 256], F32R, name="zr", tag="zr")
    nc.gpsimd.dma_start(zr[:], zf[:])

    # --- weights: 3 tiles [128,128]; rows 32n+0:18 = block-diagonal lhsT ---
    w_src = w_ap.rearrange("co ci dy dx -> dy dx ci co")
    w_tiles = []
    for dy in range(3):
        wt = wpool.tile([128, 128], F32R, name=f"w_dy{dy}", tag=f"w_dy{dy}")
        nc.sync.dma_start(wt[0:18, :], zr[0:18, 0:128])
        for g in range(2):
            for dx in range(3):
                p0 = (g * 3 + dx) * 3
                nc.gpsimd.dma_start(
                    wt[p0 : p0 + 3, g * 64 : (g + 1) * 64], w_src[dy, dx]
                )
        for n in range(1, NPER):
            nc.sync.dma_start(wt[32 * n : 32 * n + 18, :], wt[0:18, :])
        w_tiles.append(wt)

    # --- bias: [128, 1], channels replicated for both g blocks ---
    bias_t = wpool.tile([128, 1], F32, name="bias_t", tag="bias_t")
    b_src = b_ap.rearrange("(c one) -> c one", one=1)
    for g in range(2):
        nc.sync.dma_start(bias_t[g * 64 : (g + 1) * 64, :], b_src)

    # --- shift-tile slabs (2, manually alternated), all 4 images at once ---
    FS = SR * WP
    s_tiles = [
        spool.tile([128, FS], F32R, name=f"s_slab{i}", tag=f"s_slab{i}")
        for i in range(2)
    ]

    chunk_no = 0
    slab_no = 0
    for _rep in range(repeat):
        for s in range(NSLAB):
            h0 = s * SR
            st_ = s_tiles[slab_no % 2]
            slab_no += 1
            stv = st_.rearrange("p (r c) -> p r c", c=WP)
            # q-major view: svq[k] = partitions {k + 32n : n in 0..4}, step 32
            svq = st_.rearrange("(n q) f -> q n f", q=32)

            # zero pad zones (cast-free f32r copies from zr); fills then
            # overwrite the valid parts.  Union of pad columns: {0,223,224,225}.
            dma_rr().dma_start(
                stv[:, :, 0:1], zr[:, 0:SR].rearrange("p (r c) -> p r c", c=1)
            )
            dma_rr().dma_start(
                stv[:, :, 223:WP],
                zr[:, 0 : 3 * SR].rearrange("p (r c) -> p r c", c=3),
            )
            if h0 == 0:
                dma_rr().dma_start(
                    stv[:, 0:1, :], zr[:, 0:WP].rearrange("p (r c) -> p r c", r=1)
                )
            if h0 + SR == H:
                dma_rr().dma_start(
                    stv[:, SR - 1 : SR, :],
                    zr[:, 0:WP].rearrange("p (r c) -> p r c", r=1),
                )

            # --- fill S slab: 6 DMAs (g, dx), each covering all 4 images ---
            for g in range(2):
                if g == 0:
                    xr0, xr1 = max(0, h0 - 1), h0 + SR - 1
                    rl0 = xr0 - (h0 - 1)
                else:
                    xr0, xr1 = h0 + 1, min(H, h0 + SR + 1)
                    rl0 = 0
                nr = xr1 - xr0
                for dx in range(3):
                    G = g * 3 + dx
                    c0, c1 = ZCOLS[dx]
                    xc0 = c0 + dx - 1
                    for ci in range(CIN):
                        dst = svq[G * 3 + ci].rearrange("n (r c) -> n r c", c=WP)
                        dma_rr().dma_start(
                            dst[:, rl0 : rl0 + nr, c0:c1],
                            x_ap[:, ci, xr0:xr1, xc0 : xc0 + (c1 - c0)],
                        )

            # --- chunks: 14 positions x 4 images ---
            gh = h0
            for gsz in GROUPS:
                stages = [
                    stpool.tile([128, gsz * NCOL], F32, name=f"stage{n}", tag="stage")
                    for n in range(NPER)
                ]
                for j in range(gsz):
                    h = gh + j * CHROWS
                    pss = [
                        ppool.tile([128, NCOL], F32, name=f"ps{n}", tag="ps")
                        for n in range(NPER)
                    ]
                    for dy in range(3):
                        off = (h - h0 + dy) * WP
                        for n in range(NPER):
                            nc.tensor.matmul(
                                pss[n][:],
                                w_tiles[dy][32 * n : 32 * n + 18, :],
                                st_[32 * n : 32 * n + 18, off : off + NCOL],
                                start=(dy == 0),
                                stop=(dy == 2),
                                tile_position=(32 * n, 0),
                            )
                    for n in range(NPER):
                        dst = stages[n][:, j * NCOL : (j + 1) * NCOL]
                        if chunk_no % 2 == 0:
                            nc.vector.tensor_scalar_add(dst, pss[n][:], bias_t[:])
                        else:
                            nc.scalar.activation(
                                dst, pss[n][:], IDENT, bias=bias_t[:], scale=1.0
                            )
                        chunk_no += 1

                # --- output DMAs: per image, one per (g block, row-in-pair) ---
                for n in range(NPER):
                    srcv = stages[n].rearrange("p (j i w) -> p j i w", i=2, w=WP)
                    dstv = o_ap[n, :, gh : gh + gsz * CHROWS, :].rearrange(
                        "co (j g i) w -> g i co j w", g=2, i=2
                    )
                    for g in range(2):
                        for i in range(2):
                            dma_rr().dma_start(
                                dstv[g, i],
                                srcv[g * 64 : (g + 1) * 64, :, i, 0:W],
                            )
                gh += gsz * CHROWS


def build_nc(repeat=1):
    nc = bacc.Bacc("TRN2", target_bir_lowering=False, debug=False)
    x_ap = nc.dram_tensor("x", [NPER, CIN, H, W], F32R, kind="ExternalInput").ap()
    w_ap = nc.dram_tensor("weight", [COUT, CIN, KK, KK], F32R, kind="ExternalInput").ap()
    b_ap = nc.dram_tensor("bias", [COUT], F32, kind="ExternalInput").ap()
    o_ap = nc.dram_tensor("out", [NPER, COUT, H, W], F32, kind="ExternalOutput").ap()

    from contextlib import ExitStack

    with tile.TileContext(nc) as tc:
        with ExitStack() as ctx:
            _emit(ctx, tc, o_ap, x_ap, w_ap, b_ap, repeat=repeat)
    nc.compile()
    return nc


_NC_CACHE = {}


def _get_nc(repeat=1):
    if repeat not in _NC_CACHE:
        _NC_CACHE[repeat] = build_nc(repeat=repeat)
    return _NC_CACHE[repeat]


def run_cores(x, weight, bias, repeat=1):
    x = np.ascontiguousarray(np.asarray(x), dtype=np.float32)
    weight = np.ascontiguousarray(np.asarray(weight), dtype=np.float32)
    bias = np.ascontiguousarray(np.asarray(bias), dtype=np.float32)
    nc = _get_nc(repeat=repeat)
    in_maps = [
        {"x": x[c * NPER : (c + 1) * NPER], "weight": weight, "bias": bias}
        for c in range(NCORES)
    ]
    res = run_bass_kernel_spmd(nc, in_maps, list(range(NCORES))).results
    return np.concatenate([res[c]["out"] for c in range(NCORES)], axis=0)


def kernel(x, weight, bias):
    return run_cores(x, weight, bias, repeat=1)

